# revision 24
# baseline (speedup 1.0000x reference)
"""Trainium2 Bass kernel for nn_EquivariantWSSHead (gauge-equivariant GNN head).

Strategy (per sharding_hint: edges partitioned across 8 cores by dst range —
graph partitioning — so each core's segment-sum is purely local, no
collectives):

- Math reformulation: each per-edge message is a linear combination of 9
  per-src-node scalars (a 48->12 projection of x, incl. 3 self-term columns)
  with cos/sin coefficients of (t, g-t, 2t-g), derived on device from
  sin(t), sin(t/2) ACT lookups via trig identities.
- The projection table is computed on device (PE transposes + matmuls) and
  packed 4 nodes per 256-byte row (node n -> row n % 25024, slot n // 25024)
  so `dma_gather` int16 row indices cover all 100096 nodes.
- Per-edge pipeline: dma_gather src rows -> 4-way slot extraction -> trig +
  linear combination (DVE/ACT) into a persistent message stream.
- Segment sum without scatter: the host sorts each core's edges by local dst
  and lays tokens out so that scan order j maps to grid (p=j//L, c=j%L).
  A per-partition prefix scan + cross-partition offset fixup gives the global
  cumsum C; per-node sums are C[end_v] - C[start_v], fetched with one small
  dma_gather over the C stream (2 boundary tokens per node) + 16-way binary
  sub-row extraction.
- Finalize: mean-normalize, add self terms, sigmoid gate, project on (e1,e2).
"""
import sys

sys.path.insert(0, "/opt/trn_rl_repo")

import numpy as np

import concourse.bass as bass
import concourse.mybir as mybir
import concourse.tile as tile
import concourse.bacc as bacc
from concourse import bass_utils
from concourse.masks import make_identity

F32 = mybir.dt.float32
I16 = mybir.dt.int16
I8 = mybir.dt.int8
AF = mybir.ActivationFunctionType
OP = mybir.AluOpType


def _ru(x, m):
    return (x + m - 1) // m * m


class Cfg:
    def __init__(self, V, E, n_cores=8):
        assert V % (n_cores * 4) == 0
        self.V, self.E, self.NCORES = V, E, n_cores
        self.VPAD = _ru(V, 256)
        self.NT = self.VPAD // 128          # node tiles (even)
        self.TROWS = self.VPAD // 4         # gather-table rows (4 nodes/row)
        self.QMOD = self.VPAD // 4          # node n -> row n % QMOD, slot n//QMOD
        assert self.TROWS <= 32768
        self.GE = 64                        # table row width (f32) = 256B
        self.OWN = V // n_cores
        self.OWNPAD = _ru(self.OWN + 1, 128)
        self.TOWN = self.OWNPAD // 128
        self.GB = 8192                      # gather batch tokens
        # token stream: 1 leading pad + worst-case shard + slack
        worst = E // n_cores + 8 * int(np.sqrt(E / n_cores)) + 256
        self.E_PAD = _ru(worst, self.GB)
        self.NBATCH = self.E_PAD // self.GB
        self.L = self.E_PAD // 128          # scan columns per partition
        # boundary stream: 2 tokens per padded own node (B0, B1)
        self.NB_B = 2 * self.OWNPAD
        assert self.E_PAD // 16 <= 32768    # C-row index fits int16
        self.CROWS = self.E_PAD // 16       # C table rows (16 positions/row)


FULL = Cfg(100000, 1600000)

_NC_CACHE = {}


def build_nc(cfg):
    key = (cfg.V, cfg.E)
    if key in _NC_CACHE:
        return _NC_CACHE[key]
    nc = bacc.Bacc("TRN2", target_bir_lowering=False, debug=False,
                   num_devices=cfg.NCORES)

    xb = nc.dram_tensor("xb", [128, cfg.NT * 48], F32, kind="ExternalInput")
    w2 = nc.dram_tensor("w2", [96, 24], F32, kind="ExternalInput")
    gidx = nc.dram_tensor("gidx", [128, cfg.E_PAD // 16], I16, kind="ExternalInput")
    sel8 = nc.dram_tensor("sel8", [128, (cfg.E_PAD // 128) * 9], I8, kind="ExternalInput")
    ang = nc.dram_tensor("ang", [128, cfg.E_PAD // 128], F32, kind="ExternalInput")
    trf = nc.dram_tensor("trf", [128, cfg.E_PAD // 128], F32, kind="ExternalInput")
    gidxo = nc.dram_tensor("gidxo", [128, cfg.OWNPAD // 16], I16, kind="ExternalInput")
    sel8o = nc.dram_tensor("sel8o", [128, cfg.TOWN], I8, kind="ExternalInput")
    bidx = nc.dram_tensor("bidx", [128, cfg.NB_B // 16], I16, kind="ExternalInput")
    bsub = nc.dram_tensor("bsub", [128, (cfg.NB_B // 128) * 4], I8, kind="ExternalInput")
    e1b = nc.dram_tensor("e1b", [128, cfg.TOWN * 3], F32, kind="ExternalInput")
    e2b = nc.dram_tensor("e2b", [128, cfg.TOWN * 3], F32, kind="ExternalInput")

    out = nc.dram_tensor("out", [128, cfg.TOWN * 3], F32, kind="ExternalOutput")

    GE = cfg.GE
    with tile.TileContext(nc) as tc:
        with (
            tc.tile_pool(name="const", bufs=1) as cp,
            tc.tile_pool(name="dram", bufs=1, space="DRAM") as dp,
            tc.tile_pool(name="xa", bufs=2) as xap,
            tc.tile_pool(name="xt", bufs=2) as xtp,
            tc.tile_pool(name="stg", bufs=2) as stp,
            tc.tile_pool(name="psT", bufs=2, space="PSUM") as psT,
            tc.tile_pool(name="psM", bufs=2, space="PSUM") as psM,
            tc.tile_pool(name="psF", bufs=1, space="PSUM") as psF,
            tc.tile_pool(name="gth", bufs=3) as gp,
            tc.tile_pool(name="edg", bufs=2) as edp,
            tc.tile_pool(name="trg", bufs=2) as trp,
            tc.tile_pool(name="stream", bufs=1) as smp,
            tc.tile_pool(name="fin", bufs=1) as fp,
        ):
            ident = cp.tile([128, 128], F32)
            make_identity(nc, ident[:])
            w2_t = cp.tile([96, 24], F32)
            nc.sync.dma_start(out=w2_t[:], in_=w2.ap())
            pi_t = cp.tile([128, 1], F32)
            nc.vector.memset(pi_t[:], np.pi)

            table = dp.tile([cfg.TROWS, GE], F32)
            ctab = dp.tile([cfg.CROWS, GE], F32)  # cumsum stream as 256B rows

            # zero the 16 unused tail columns of every table row (gathered
            # bytes must be defined; compute never reads them)
            zpad = cp.tile([128, 196 * 16], F32)
            nc.vector.memset(zpad[:], 0.0)
            nfull = cfg.TROWS // 128 * 128
            rpp = nfull // 128
            if rpp > 0:
                dst0 = bass.AP(table[:].tensor, 48,
                               [[rpp * GE, 128], [GE, rpp], [1, 16]])
                nc.sync.dma_start(out=dst0, in_=zpad[:, : rpp * 16])
            tail = cfg.TROWS - nfull
            if tail > 0:
                dst1 = bass.AP(table[:].tensor, nfull * GE + 48,
                               [[GE, tail], [1, 16]])
                nc.sync.dma_start(out=dst1, in_=zpad[: tail, :16])

            # ---------- Phase A: node projection table ----------
            CH = 32  # tiles per chunk
            t0 = 0
            while t0 < cfg.NT:
                nt = min(CH, cfg.NT - t0)
                sfx = "" if nt == CH else "T"
                xc = xap.tile([128, nt * 48], F32, tag="xc" + sfx)
                nc.sync.dma_start(
                    out=xc[:, : nt * 48],
                    in_=xb.ap()[:, t0 * 48:(t0 + nt) * 48],
                )
                stg = stp.tile([128, nt * 12], F32, tag="stg" + sfx)
                pM = psM.tile([128, CH * 12], F32, tag="pM")
                npair = nt // 2
                for pg in range(0, npair, 4):
                    pe = min(pg + 4, npair)
                    pT = psT.tile([96, 512], F32, tag="pT")
                    for p in range(pg, pe):
                        nc.tensor.transpose(
                            out=pT[:, (p - pg) * 128:(p - pg + 1) * 128],
                            in_=xc[:, p * 96:(p + 1) * 96],
                            identity=ident[:],
                        )
                    xt = xtp.tile([96, 512], F32, tag="xt")
                    if (pg // 4) % 2 == 0:
                        nc.vector.tensor_copy(out=xt[:, : (pe - pg) * 128],
                                              in_=pT[:, : (pe - pg) * 128])
                    else:
                        nc.scalar.copy(out=xt[:, : (pe - pg) * 128],
                                       in_=pT[:, : (pe - pg) * 128])
                    for p in range(pg, pe):
                        nc.tensor.matmul(
                            out=pM[:, p * 24:(p + 1) * 24],
                            lhsT=xt[:, (p - pg) * 128:(p - pg + 1) * 128],
                            rhs=w2_t[:],
                            start=True, stop=True,
                        )
                nc.vector.tensor_copy(out=stg[:, : nt * 12], in_=pM[:, : nt * 12])
                # store: node n = (t0+t)*128 + p -> table row n % QMOD, in-row
                # offset (n // QMOD)*12, split at quarter boundaries
                pieces = []
                n_lo, n_hi = t0 * 128, (t0 + nt) * 128
                q_lo, q_hi = n_lo // cfg.QMOD, (n_hi - 1) // cfg.QMOD
                for q in range(q_lo, q_hi + 1):
                    a = max(n_lo, q * cfg.QMOD)
                    bnd = min(n_hi, (q + 1) * cfg.QMOD)
                    pieces.append((q, a, bnd))
                for (q, a, bnd) in pieces:
                    def box(tt, pa, pb, ntt):
                        row0 = (tt * 128 + pa) % cfg.QMOD
                        dstap = bass.AP(
                            table[:].tensor,
                            row0 * GE + q * 12,
                            [[GE, pb - pa], [128 * GE, ntt], [1, 12]],
                        )
                        srcap = stg[pa:pb,
                                    (tt - t0) * 12:(tt - t0 + ntt) * 12].rearrange(
                                        "p (t u) -> p t u", u=12)
                        nc.sync.dma_start(out=dstap, in_=srcap)
                    pos = a
                    if pos % 128 != 0:
                        tt = pos // 128
                        pa = pos % 128
                        pb = min(128, bnd - tt * 128)
                        box(tt, pa, pb, 1)
                        pos = tt * 128 + pb
                    nwhole = (bnd - pos) // 128
                    if nwhole > 0:
                        box(pos // 128, 0, 128, nwhole)
                        pos += nwhole * 128
                    if pos < bnd:
                        box(pos // 128, 0, bnd - pos, 1)
                        pos = bnd
                t0 += nt

            # ---------- own-node self terms ----------
            gio = fp.tile([128, cfg.OWNPAD // 16], I16)
            nc.sync.dma_start(out=gio[:], in_=gidxo.ap())
            selo = fp.tile([128, cfg.TOWN], I8)
            nc.sync.dma_start(out=selo[:], in_=sel8o.ap())
            gto = fp.tile([128, cfg.TOWN * GE], F32)
            go3 = gto[:].rearrange("p (b e) -> p b e", e=GE)
            pos = 0
            while pos < cfg.OWNPAD:
                n = min(cfg.GB, cfg.OWNPAD - pos)
                nc.gpsimd.dma_gather(
                    out_ap=go3[:, pos // 128:(pos + n) // 128, :],
                    in_ap=table[:],
                    idxs_ap=gio[:, pos // 16:(pos + n) // 16],
                    num_idxs=n, num_idxs_reg=n, elem_size=GE,
                    single_packet=False,
                )
                pos += n
            xo3 = go3[:, :, 9:12]
            for k in (1, 2, 3):
                mk = fp.tile([128, cfg.TOWN], I8, tag=f"mko{k}")
                nc.vector.tensor_scalar(out=mk[:], in0=selo[:], scalar1=k,
                                        scalar2=None, op0=OP.is_equal)
                nc.vector.copy_predicated(
                    out=xo3, mask=mk[:].to_broadcast([128, cfg.TOWN, 3]),
                    data=go3[:, :, 12 * k + 9:12 * k + 12])

            # persistent message stream [128, L, 4]
            msg = smp.tile([128, cfg.L * 4], F32)
            m4 = msg[:].rearrange("p (c e) -> p c e", e=4)

            # ---------- Phase B: edge batches ----------
            NBL = cfg.GB // 128  # token columns per batch (64)
            for b in range(cfg.NBATCH):
                gi = edp.tile([128, cfg.GB // 16], I16, tag="gi")
                nc.sync.dma_start(
                    out=gi[:], in_=gidx.ap()[:, b * (cfg.GB // 16):(b + 1) * (cfg.GB // 16)])
                sel = edp.tile([128, NBL * 9], I8, tag="sel")
                nc.sync.dma_start(out=sel[:], in_=sel8.ap()[:, b * NBL * 9:(b + 1) * NBL * 9])
                an = edp.tile([128, NBL], F32, tag="an")
                nc.sync.dma_start(out=an[:], in_=ang.ap()[:, b * NBL:(b + 1) * NBL])
                tr = edp.tile([128, NBL], F32, tag="tr")
                nc.sync.dma_start(out=tr[:], in_=trf.ap()[:, b * NBL:(b + 1) * NBL])

                gt = gp.tile([128, NBL * GE], F32, tag="gt")
                nc.gpsimd.dma_gather(
                    out_ap=gt[:].rearrange("p (b e) -> p b e", e=GE),
                    in_ap=table[:],
                    idxs_ap=gi[:],
                    num_idxs=cfg.GB, num_idxs_reg=cfg.GB, elem_size=GE,
                    single_packet=False,
                )
                g3 = gt[:].rearrange("p (b e) -> p b e", e=GE)

                # 4-way slot extraction: ext = sum_k (sel==k) * slot_k
                # sel arrives pre-replicated 9x per token (no broadcast APs)
                ext = trp.tile([128, NBL * 9], F32, tag="ext")
                e3 = ext[:].rearrange("p (b u) -> p b u", u=9)
                mk = trp.tile([128, NBL * 9], F32, tag="mkf")
                mk3 = mk[:].rearrange("p (b u) -> p b u", u=9)
                tmp9 = trp.tile([128, NBL * 9], F32, tag="tmp9")
                t93 = tmp9[:].rearrange("p (b u) -> p b u", u=9)
                nc.vector.tensor_scalar(out=mk[:], in0=sel[:], scalar1=0,
                                        scalar2=None, op0=OP.is_equal)
                nc.vector.tensor_tensor(out=e3, in0=g3[:, :, 0:9], in1=mk3,
                                        op=OP.mult)
                for k in (1, 2, 3):
                    nc.vector.tensor_scalar(out=mk[:], in0=sel[:], scalar1=k,
                                            scalar2=None, op0=OP.is_equal)
                    nc.vector.tensor_tensor(out=t93, in0=g3[:, :, 12 * k:12 * k + 9],
                                            in1=mk3, op=OP.mult)
                    nc.vector.tensor_tensor(out=ext[:], in0=ext[:], in1=tmp9[:],
                                            op=OP.add)

                # trig via identities (ACT Sin domain is [-pi, pi])
                def sin_full(nm, src_t, scale):
                    tt = trp.tile([128, NBL], F32, tag=nm)
                    nc.scalar.activation(tt[:], src_t[:], AF.Sin,
                                         bias=pi_t[:], scale=scale)
                    return tt

                def cos_from_half(nm, half):
                    tt = trp.tile([128, NBL], F32, tag=nm)
                    nc.vector.tensor_tensor(out=tt[:], in0=half[:], in1=half[:],
                                            op=OP.mult)
                    nc.vector.tensor_scalar(out=tt[:], in0=tt[:], scalar1=-2.0,
                                            scalar2=1.0, op0=OP.mult, op1=OP.add)
                    return tt

                st = sin_full("st", an, -1.0)
                st2 = sin_full("st2", an, -0.5)
                ct = cos_from_half("ct", st2)
                sg = sin_full("sg", tr, -1.0)
                sg2 = sin_full("sg2", tr, -0.5)
                cg = cos_from_half("cg", sg2)

                def tt_op(nm, a, bb, op):
                    o = trp.tile([128, NBL], F32, tag=nm)
                    nc.vector.tensor_tensor(out=o[:], in0=a[:], in1=bb[:], op=op)
                    return o

                pA = tt_op("pA", cg, ct, OP.mult)
                pB = tt_op("pB", sg, st, OP.mult)
                cd = tt_op("cd", pA, pB, OP.add)
                pC = tt_op("pC", sg, ct, OP.mult)
                pD = tt_op("pD", cg, st, OP.mult)
                sd = tt_op("sd", pC, pD, OP.subtract)
                c2 = cos_from_half("c2", st)
                s2 = trp.tile([128, NBL], F32, tag="s2")
                nc.vector.scalar_tensor_tensor(out=s2[:], in0=st[:], scalar=2.0,
                                               in1=ct[:], op0=OP.mult, op1=OP.mult)
                qA = tt_op("qA", c2, cg, OP.mult)
                qB = tt_op("qB", s2, sg, OP.mult)
                chv = tt_op("chv", qA, qB, OP.add)
                qC = tt_op("qC", s2, cg, OP.mult)
                qD = tt_op("qD", c2, sg, OP.mult)
                shv = tt_op("shv", qC, qD, OP.subtract)

                def ch_(c):
                    return e3[:, :, c]

                m3 = m4[:, b * NBL:(b + 1) * NBL, :]
                tA = trp.tile([128, NBL], F32, tag="tA")
                tB = trp.tile([128, NBL], F32, tag="tB")

                def mul(o, a, bb):
                    nc.vector.tensor_tensor(out=o, in0=a, in1=bb, op=OP.mult)

                def add(o, a, bb):
                    nc.vector.tensor_tensor(out=o, in0=a, in1=bb, op=OP.add)

                def sub(o, a, bb):
                    nc.vector.tensor_tensor(out=o, in0=a, in1=bb, op=OP.subtract)

                # m0 = na + cd*zr - sd*zi
                mul(tA[:], cd[:], ch_(1))
                mul(tB[:], sd[:], ch_(2))
                sub(tA[:], tA[:], tB[:])
                add(m3[:, :, 0], tA[:], ch_(0))
                # mv1 = ct*sa - st*sb + cg*pr - sg*pi + ch*rr - sh*ri
                mul(tA[:], ct[:], ch_(3))
                mul(tB[:], st[:], ch_(4))
                sub(tA[:], tA[:], tB[:])
                mul(tB[:], cg[:], ch_(5))
                add(tA[:], tA[:], tB[:])
                mul(tB[:], sg[:], ch_(6))
                sub(tA[:], tA[:], tB[:])
                mul(tB[:], chv[:], ch_(7))
                add(tA[:], tA[:], tB[:])
                mul(tB[:], shv[:], ch_(8))
                sub(m3[:, :, 1], tA[:], tB[:])
                # mv2 = st*sa + ct*sb + sg*pr + cg*pi + sh*rr + ch*ri
                mul(tA[:], st[:], ch_(3))
                mul(tB[:], ct[:], ch_(4))
                add(tA[:], tA[:], tB[:])
                mul(tB[:], sg[:], ch_(5))
                add(tA[:], tA[:], tB[:])
                mul(tB[:], cg[:], ch_(6))
                add(tA[:], tA[:], tB[:])
                mul(tB[:], shv[:], ch_(7))
                add(tA[:], tA[:], tB[:])
                mul(tB[:], chv[:], ch_(8))
                add(m3[:, :, 2], tA[:], tB[:])
                # deg component = 1.0 (ct*0 + 1; avoids a strided memset)
                nc.vector.tensor_scalar(out=m3[:, :, 3], in0=ct[:], scalar1=0.0,
                                        scalar2=1.0, op0=OP.mult, op1=OP.add)

            # token at scan position 0 is the cumsum baseline: zero it
            nc.vector.memset(msg[0:1, 0:4], 0.0)

            # ---------- scan: per-partition inclusive cumsum + offsets ----
            for c in range(4):
                v = msg[:, c::4]
                nc.vector.tensor_tensor_scan(
                    out=v, data0=v, data1=v, initial=0.0,
                    op0=OP.add, op1=OP.bypass)
            # per-partition totals -> exclusive offsets across partitions
            tot = fp.tile([128, 4], F32)
            nc.vector.tensor_copy(out=tot[:], in_=msg[:, (cfg.L - 1) * 4:cfg.L * 4])
            pTot = psF.tile([4, 128], F32, tag="pTot")
            nc.tensor.transpose(out=pTot[:], in_=tot[:], identity=ident[:])
            totT = fp.tile([4, 128], F32)
            nc.vector.tensor_copy(out=totT[:], in_=pTot[:])
            scT = fp.tile([4, 128], F32)
            nc.vector.tensor_tensor_scan(
                out=scT[:], data0=totT[:], data1=totT[:], initial=0.0,
                op0=OP.add, op1=OP.bypass)
            nc.vector.tensor_tensor(out=scT[:], in0=scT[:], in1=totT[:],
                                    op=OP.subtract)  # exclusive
            pOff = psF.tile([128, 4], F32, tag="pOff")
            nc.tensor.transpose(out=pOff[:], in_=scT[:], identity=ident[0:4, 0:4])
            off = fp.tile([128, 4], F32)
            nc.vector.tensor_copy(out=off[:], in_=pOff[:])
            for c in range(4):
                nc.vector.tensor_scalar(
                    out=msg[:, c::4], in0=msg[:, c::4],
                    scalar1=off[:, c:c + 1], scalar2=None, op0=OP.add)

            # store C stream to DRAM: token j = p*L + c at flat j*4
            cflat = bass.AP(ctab[:].tensor, 0,
                            [[cfg.L * 4, 128], [1, cfg.L * 4]])
            nc.sync.dma_start(out=cflat, in_=msg[:])

            # ---------- boundary gather: B0/B1 per own node ----------
            gib = fp.tile([128, cfg.NB_B // 16], I16)
            nc.sync.dma_start(out=gib[:], in_=bidx.ap())
            bsu = fp.tile([128, (cfg.NB_B // 128) * 4], I8)
            nc.sync.dma_start(out=bsu[:], in_=bsub.ap())
            NBC = cfg.NB_B // 128            # boundary token columns (2*TOWN)
            bval = fp.tile([128, NBC * 4], F32)
            bv3 = bval[:].rearrange("p (b e) -> p b e", e=4)
            pos = 0
            while pos < cfg.NB_B:
                n = min(cfg.GB, cfg.NB_B - pos)
                ncol = n // 128
                c0 = pos // 128
                gt = gp.tile([128, (cfg.GB // 128) * GE], F32, tag="gt")
                nc.gpsimd.dma_gather(
                    out_ap=gt[:, : ncol * GE].rearrange("p (b e) -> p b e", e=GE),
                    in_ap=ctab[:],
                    idxs_ap=gib[:, pos // 16:(pos + n) // 16],
                    num_idxs=n, num_idxs_reg=n, elem_size=GE,
                    single_packet=False,
                )
                gb4 = gt[:, : ncol * GE].rearrange(
                    "p (b s e) -> p b s e", s=16, e=4)
                # 16-way extraction: acc = sum_s (bsub==s) * sub_s
                bm = fp.tile([128, (cfg.GB // 128) * 4], F32, tag="bbm")
                bm3 = bm[:, : ncol * 4].rearrange("p (b e) -> p b e", e=4)
                bt = fp.tile([128, (cfg.GB // 128) * 4], F32, tag="bbt")
                bt3 = bt[:, : ncol * 4].rearrange("p (b e) -> p b e", e=4)
                bsl = bsu[:, c0 * 4:(c0 + ncol) * 4]
                for s in range(16):
                    nc.vector.tensor_scalar(
                        out=bm[:, : ncol * 4], in0=bsl,
                        scalar1=s, scalar2=None, op0=OP.is_equal)
                    if s == 0:
                        nc.vector.tensor_tensor(
                            out=bv3[:, c0:c0 + ncol, :], in0=gb4[:, :, 0, :],
                            in1=bm3, op=OP.mult)
                    else:
                        nc.vector.tensor_tensor(out=bt3, in0=gb4[:, :, s, :],
                                                in1=bm3, op=OP.mult)
                        nc.vector.tensor_tensor(
                            out=bv3[:, c0:c0 + ncol, :],
                            in0=bv3[:, c0:c0 + ncol, :], in1=bt3, op=OP.add)
                pos += n
            # per-node sums: B1 - B0  ([128, TOWN, 4])
            b0 = bv3[:, 0:cfg.TOWN, :]
            b1 = bv3[:, cfg.TOWN:2 * cfg.TOWN, :]
            acc = fp.tile([128, cfg.TOWN * 4], F32)
            a3 = acc[:].rearrange("p (b e) -> p b e", e=4)
            nc.vector.tensor_tensor(out=a3, in0=b1, in1=b0, op=OP.subtract)

            # ---------- finalize ----------
            deg = fp.tile([128, cfg.TOWN], F32)
            nc.vector.tensor_scalar(out=deg[:], in0=a3[:, :, 3], scalar1=1.0,
                                    scalar2=None, op0=OP.max)
            inv = fp.tile([128, cfg.TOWN], F32)
            nc.vector.reciprocal(out=inv[:], in_=deg[:])

            e1t = fp.tile([128, cfg.TOWN * 3], F32)
            nc.sync.dma_start(out=e1t[:], in_=e1b.ap())
            e2t = fp.tile([128, cfg.TOWN * 3], F32)
            nc.sync.dma_start(out=e2t[:], in_=e2b.ap())

            mag = fp.tile([128, cfg.TOWN], F32)
            nc.vector.tensor_tensor(out=mag[:], in0=a3[:, :, 0], in1=inv[:], op=OP.mult)
            nc.vector.tensor_tensor(out=mag[:], in0=mag[:], in1=xo3[:, :, 0], op=OP.add)
            t1 = fp.tile([128, cfg.TOWN], F32)
            nc.vector.tensor_tensor(out=t1[:], in0=a3[:, :, 1], in1=inv[:], op=OP.mult)
            nc.vector.tensor_tensor(out=t1[:], in0=t1[:], in1=xo3[:, :, 1], op=OP.add)
            t2 = fp.tile([128, cfg.TOWN], F32)
            nc.vector.tensor_tensor(out=t2[:], in0=a3[:, :, 2], in1=inv[:], op=OP.mult)
            nc.vector.tensor_tensor(out=t2[:], in0=t2[:], in1=xo3[:, :, 2], op=OP.add)
            sgm = fp.tile([128, cfg.TOWN], F32)
            nc.scalar.activation(sgm[:], mag[:], AF.Sigmoid)

            ot = fp.tile([128, cfg.TOWN * 3], F32)
            o3 = ot[:].rearrange("p (b u) -> p b u", u=3)
            e13 = e1t[:].rearrange("p (b u) -> p b u", u=3)
            e23 = e2t[:].rearrange("p (b u) -> p b u", u=3)
            tX = fp.tile([128, cfg.TOWN], F32, tag="tX")
            for j in range(3):
                nc.vector.tensor_tensor(out=o3[:, :, j], in0=t1[:], in1=e13[:, :, j], op=OP.mult)
                nc.vector.tensor_tensor(out=tX[:], in0=t2[:], in1=e23[:, :, j], op=OP.mult)
                nc.vector.tensor_tensor(out=o3[:, :, j], in0=o3[:, :, j], in1=tX[:], op=OP.add)
                nc.vector.tensor_tensor(out=o3[:, :, j], in0=o3[:, :, j], in1=sgm[:], op=OP.mult)
            nc.sync.dma_start(out=out.ap(), in_=ot[:])

    nc.finalize()
    _NC_CACHE[key] = nc
    return nc


def _wrap16(tok, epad):
    a = np.zeros(epad, dtype=np.int16)
    a[: len(tok)] = tok
    a = a.reshape(epad // 16, 16).T.copy()       # token i -> [i%16, i//16]
    return np.tile(a, (8, 1))


def _toklay(v, epad, fill=0.0, dtype=np.float32):
    a = np.full(epad, fill, dtype=dtype)
    a[: len(v)] = v
    return a.reshape(epad // 128, 128).T.copy()  # token i -> [i%128, i//128]


def pack_inputs(cfg, x, edge_index, angles, transporters, e1, e2,
                w_self0, w_n00, w_n10, w_self11, w_n01, w_n11):
    V = cfg.V
    C0 = C1 = 16
    W = np.zeros((48, 12), dtype=np.float32)
    w10a, w10b = w_n10[:, 0], w_n10[:, 1]
    p_, q_, r_, s_ = w_n11[:, 0], w_n11[:, 1], w_n11[:, 2], w_n11[:, 3]
    sa_, sb_ = w_self11[:, 0], w_self11[:, 1]
    k = np.arange(C1)
    a1i, a2i = 16 + 2 * k, 17 + 2 * k
    W[a1i, 1] = w10a; W[a2i, 1] = w10b
    W[a2i, 2] = w10a; W[a1i, 2] = -w10b
    W[a1i, 5] = p_;   W[a2i, 5] = -q_
    W[a2i, 6] = p_;   W[a1i, 6] = q_
    W[a1i, 7] = r_;   W[a2i, 7] = s_
    W[a1i, 8] = s_;   W[a2i, 8] = -r_
    W[a1i, 10] = sa_; W[a2i, 10] = -sb_
    W[a2i, 11] = sa_; W[a1i, 11] = sb_
    W[:C0, 0] = w_n00
    W[:C0, 3] = w_n01[:, 0]
    W[:C0, 4] = w_n01[:, 1]
    W[:C0, 9] = w_self0
    W2 = np.zeros((96, 24), dtype=np.float32)
    W2[:48, :12] = W
    W2[48:, 12:] = W

    xpad = np.zeros((cfg.VPAD, 48), dtype=np.float32)
    xpad[:V] = x
    xb = xpad.reshape(cfg.NT, 128, 48).transpose(1, 0, 2).reshape(128, -1).copy()

    src = np.asarray(edge_index[0]).astype(np.int64)
    dst = np.asarray(edge_index[1]).astype(np.int64)
    ang = np.asarray(angles).astype(np.float32)
    trf = np.asarray(transporters).astype(np.float32)

    # token i (gather layout) <-> scan position j: j = (i%128)*L + i//128
    epad = cfg.E_PAD
    L = cfg.L
    i_all = np.arange(epad)
    j_of_i = (i_all % 128) * L + i_all // 128

    in_maps = []
    for c in range(cfg.NCORES):
        lo, hi = c * cfg.OWN, (c + 1) * cfg.OWN
        ids = np.nonzero((dst >= lo) & (dst < hi))[0]
        dl = (dst[ids] - lo).astype(np.int64)
        order = np.argsort(dl, kind="stable")
        eidx = ids[order]
        dls = dl[order]
        n = len(eidx)
        if n + 1 > epad:
            raise RuntimeError("edge shard exceeds E_PAD")
        # scan-position arrays (position 0 is the zero baseline pad)
        gj = np.zeros(epad, dtype=np.int16)
        sj = np.zeros(epad, dtype=np.int8)
        aj = np.zeros(epad, dtype=np.float32)
        tj = np.zeros(epad, dtype=np.float32)
        gj[1:n + 1] = (src[eidx] % cfg.QMOD).astype(np.int16)
        sj[1:n + 1] = (src[eidx] // cfg.QMOD).astype(np.int8)
        aj[1:n + 1] = ang[eidx]
        tj[1:n + 1] = trf[eidx]
        # reorder scan-position arrays into token order
        g_tok = gj[j_of_i]
        s_tok = sj[j_of_i]
        a_tok = aj[j_of_i]
        t_tok = tj[j_of_i]

        # boundaries: inclusive-cumsum positions per node (scan positions)
        rowptr = np.searchsorted(dls, np.arange(cfg.OWN + 1))  # 0..n
        b0 = np.zeros(cfg.OWNPAD, dtype=np.int64)
        b1 = np.zeros(cfg.OWNPAD, dtype=np.int64)
        b0[: cfg.OWN] = rowptr[:-1]        # C at last pos before v's run
        b1[: cfg.OWN] = rowptr[1:]         # C at last pos of v's run
        btok = np.concatenate([b0, b1])
        bidx_np = _wrap16((btok // 16).astype(np.int16), cfg.NB_B)
        bsub_np = np.repeat(_toklay((btok % 16).astype(np.int8), cfg.NB_B, 0, np.int8), 4, axis=1)

        n_own = lo + np.arange(cfg.OWNPAD)
        n_own = np.minimum(n_own, V - 1)
        gidxo = _wrap16((n_own % cfg.QMOD).astype(np.int16), cfg.OWNPAD)
        sel8o = _toklay((n_own // cfg.QMOD).astype(np.int8), cfg.OWNPAD, 0, np.int8)

        def blk(a):
            return a.reshape(cfg.TOWN, 128, 3).transpose(1, 0, 2).reshape(128, -1).copy()

        e1p = np.zeros((cfg.OWNPAD, 3), dtype=np.float32)
        e1p[: cfg.OWN] = 2.0 * np.asarray(e1[lo:hi], dtype=np.float32)
        e2p = np.zeros((cfg.OWNPAD, 3), dtype=np.float32)
        e2p[: cfg.OWN] = 2.0 * np.asarray(e2[lo:hi], dtype=np.float32)

        in_maps.append({
            "xb": xb, "w2": W2,
            "gidx": _wrap16(g_tok, epad),
            "sel8": np.repeat(_toklay(s_tok, epad, 0, np.int8), 9, axis=1),
            "ang": _toklay(a_tok, epad),
            "trf": _toklay(t_tok, epad),
            "gidxo": gidxo, "sel8o": sel8o,
            "bidx": bidx_np, "bsub": bsub_np,
            "e1b": blk(e1p), "e2b": blk(e2p),
        })
    return in_maps


def unshard(cfg, results):
    out = np.zeros((cfg.V, 3), dtype=np.float32)
    for c, res in enumerate(results):
        o = res["out"].reshape(128, cfg.TOWN, 3).transpose(1, 0, 2).reshape(-1, 3)
        out[c * cfg.OWN:(c + 1) * cfg.OWN] = o[: cfg.OWN]
    return out


def kernel(**inputs):
    cfg = FULL
    nc = build_nc(cfg)
    in_maps = pack_inputs(cfg, **inputs)
    res = bass_utils.run_bass_kernel_spmd(
        nc, in_maps, core_ids=list(range(cfg.NCORES)))
    return unshard(cfg, [r for r in res.results])
